# revision 14
# baseline (speedup 1.0000x reference)
"""AttentiveMatch kernel for Trainium2 (8 NeuronCores, data-parallel over batch).

Reference math (per batch):
    pn = l2norm(p); qn = l2norm(q)
    w  = -(pn @ qn^T) / D          # [S,S]
    mv = (w @ q) / S               # [S,D]
    mn = l2norm(mv)
    out = -mean(pn * mn, -1)       # [S]

Rewritten with u_i = sum_j cos_ij q_j (signs cancel):
    out_i = (1/D) (pn_i . u_i) / |u_i|

Device pipeline (fp8 DoubleRow matmuls, all scalar folding done on host):
    mm1 [j,i]:  C' = (s2 q)^T-layout.T @ (s1 pn)^T-layout = s1 s2 |q_j| cos
    A8 = fp8(C' * G/(s1 s2))                 DVE tensor_scalar, const scale
    mm2 [i,d]:  M2 = A8-lhsT.T @ (s1 qn) = s1 G u    (i-partition layout)
    ss col  = accum_out of ACT Square(M2/256)        free-axis reduce
    dot col = accum_out of DVE (pn8 * M2) / 4096     tensor_tensor_reduce
Both land as [i%128, b*ST+it] columns; host does out = dot / (D sqrt(ss)).
Norms |p|,|q| folded into the host-side fp8 operands.
"""

import os
import sys

for _p in ("/opt/trn_rl_repo",):
    if _p not in sys.path:
        sys.path.append(_p)

import numpy as np
import ml_dtypes

import concourse.bacc as bacc
import concourse.mybir as mybir
import concourse.tile as tile
from concourse.bass_utils import run_bass_kernel_spmd

B, S, D = 64, 512, 768
NCORES = 8
BP = B // NCORES          # batches per core
ST = S // 128             # s tiles (4)
KT = D // 128             # d tiles (6)
F32 = mybir.dt.float32
BF16 = mybir.dt.bfloat16
F8 = mybir.dt.float8e4
AF = mybir.ActivationFunctionType
ALU = mybir.AluOpType
DR = mybir.MatmulPerfMode.DoubleRow

S1 = 16.0                 # fp8 pre-scale for pn/qn operands
S2 = 8.0                  # fp8 pre-scale for the raw-q operand
G = 16.0                  # fp8 A-matrix scale; S1*G = 256 so ss = |u|^2
C_A = G / (S1 * S2)       # PSUM -> A8
C_S = 1.0 / (S1 * G)      # M2 -> u
C_D = 1.0 / (S1 * S1 * G)  # (s1 pn)*(M2) -> pn.u

_NC = None

if os.environ.get("KERNEL_LDW_OPT", "0") == "1":
    import concourse.bass_utils as _bu

    _orig_run_command = _bu.run_command

    def _patched_run_command(cmd, **kw):
        cmd = [
            ("--enable-ldw-opt=true" if c == "--enable-ldw-opt=false" else c)
            for c in cmd
        ]
        return _orig_run_command(cmd, **kw)

    _bu.run_command = _patched_run_command


def _build():
    nc = bacc.Bacc("TRN2", target_bir_lowering=False, debug=False, num_devices=NCORES)
    qt_d = nc.dram_tensor("qt", [BP, 128, KT, S], F8, kind="ExternalInput")
    pt_d = nc.dram_tensor("pt", [BP, 128, KT, S], F8, kind="ExternalInput")
    qn_d = nc.dram_tensor("qn", [BP, 128, ST, D], F8, kind="ExternalInput")
    pn_d = nc.dram_tensor("pn", [BP, 128, ST, D], BF16, kind="ExternalInput")
    od_d = nc.dram_tensor("od", [128, BP * ST], F32, kind="ExternalOutput")
    os_d = nc.dram_tensor("os", [128, BP * ST], F32, kind="ExternalOutput")

    with tile.TileContext(nc) as tc:
        with (
            tc.tile_pool(name="cst", bufs=1) as cst,
            tc.tile_pool(name="inp", bufs=3) as inp,
            tc.tile_pool(name="ats", bufs=2) as ats,
            tc.tile_pool(name="gps", bufs=3, space="PSUM") as gps,
            tc.tile_pool(name="mps", bufs=2, space="PSUM") as mps,
            tc.tile_pool(name="scr", bufs=2) as scr,
        ):
            out_dot = cst.tile([128, BP * ST], F32)
            out_ss = cst.tile([128, BP * ST], F32)

            for b in range(BP):
                qt = inp.tile([128, KT, S], F8, tag="qt")
                pt = inp.tile([128, KT, S], F8, tag="pt")
                for c in range(KT // 2):
                    sl = slice(2 * c, 2 * c + 2)
                    nc.sync.dma_start(qt[:, sl, :], qt_d[b, :, sl, :])
                    nc.gpsimd.dma_start(pt[:, sl, :], pt_d[b, :, sl, :])
                qn = inp.tile([128, ST, D], F8, tag="qn")
                nc.gpsimd.dma_start(qn[:], qn_d[b])
                pnN = inp.tile([128, ST, D], BF16, tag="pn")
                nc.scalar.dma_start(pnN[:], pn_d[b])

                # mm1: C'[j,i] = sum_d (s2 q)[j,d] (s1 pn)[i,d]; A8 peel on
                # DVE with a constant scale (|q| folded into the qt operand).
                at_all = ats.tile([128, ST, S], F8, tag="at")
                for j in range(ST):
                    g_ps = gps.tile([128, S], F32, tag="g")
                    for c in range(KT // 2):
                        nc.tensor.matmul(
                            g_ps[:],
                            lhsT=qt[:, 2 * c:2 * c + 2, j * 128:(j + 1) * 128],
                            rhs=pt[:, 2 * c:2 * c + 2, :],
                            start=(c == 0), stop=(c == KT // 2 - 1),
                            perf_mode=DR,
                        )
                    nc.vector.tensor_scalar_mul(at_all[:, j, :], g_ps[:], C_A)

                # mm2 (i-partition layout): M2[i,d] = sum_j A8[j,i] (s1 qn)[j,d]
                # d split 512+256 to respect the one-bank matmul-out rule.
                # ss_i  = sum_d (M2/256)^2      ACT Square + accum_out
                # dot_i = sum_d pn8 M2 / 4096   DVE tensor_tensor_reduce
                # Both accum_outs land in the final [i%128, it] layout.
                sq = scr.tile([128, D], BF16, tag="sq")
                dp = scr.tile([128, D], BF16, tag="dp")
                for it in range(ST):
                    col = slice(b * ST + it, b * ST + it + 1)
                    m2 = mps.tile([128, 1024], F32, tag="m2")
                    lh = [at_all[:, 2 * c:2 * c + 2, it * 128:(it + 1) * 128]
                          for c in range(ST // 2)]
                    for lo, hi in ((0, 512), (512, 768)):
                        for c in range(ST // 2):
                            nc.tensor.matmul(
                                m2[:, lo:hi],
                                lhsT=lh[c],
                                rhs=qn[:, 2 * c:2 * c + 2, lo:hi],
                                start=(c == 0), stop=(c == ST // 2 - 1),
                                perf_mode=DR,
                            )
                    nc.scalar.activation(
                        sq[:], m2[:, 0:D], AF.Square, scale=C_S,
                        accum_out=out_ss[:, col])
                    nc.vector.tensor_mul(dp[:], pnN[:, it, :], m2[:, 0:D])
                    nc.vector.tensor_reduce(out_dot[:, col], dp[:],
                                            axis=mybir.AxisListType.X,
                                            op=ALU.add)

            nc.sync.dma_start(od_d[:], out_dot[:])
            nc.sync.dma_start(os_d[:], out_ss[:])
    nc.compile()
    return nc


def _get_nc():
    global _NC
    if _NC is None:
        _NC = _build()
    return _NC


def _prep_inputs(p, q):
    p = np.asarray(p, dtype=np.float32)
    q = np.asarray(q, dtype=np.float32)
    nq = np.sqrt(np.maximum((q * q).sum(-1), 1e-12))          # [B,S]
    npn = np.sqrt(np.maximum((p * p).sum(-1), 1e-12))
    q8 = (q * S2).astype(ml_dtypes.float8_e4m3)               # s2 * q
    pn8 = (p * (S1 / npn[..., None])).astype(ml_dtypes.float8_e4m3)
    qn8 = (q * (S1 / nq[..., None])).astype(ml_dtypes.float8_e4m3)

    # transposed: [core, b, part, kt, s] with d = kt*128 + part
    def tr(x):
        return np.ascontiguousarray(
            x.reshape(NCORES, BP, S, KT, 128).transpose(0, 1, 4, 3, 2)
        )

    # natural: [core, b, part, jt, d] with s = jt*128 + part
    def nat(x):
        return np.ascontiguousarray(
            x.reshape(NCORES, BP, ST, 128, D).transpose(0, 1, 3, 2, 4)
        )

    qt, pt = tr(q8), tr(pn8)
    qn_nat = nat(qn8)
    pn_nat = nat(pn8.astype(ml_dtypes.bfloat16))
    return [
        {"qt": qt[c], "pt": pt[c], "qn": qn_nat[c], "pn": pn_nat[c]}
        for c in range(NCORES)
    ]


def _postprocess(results):
    # [core, part, b*ST+it] -> batch core*BP+b, i = it*128 + part
    def unpack(key):
        x = np.stack([np.asarray(r[key], dtype=np.float32) for r in results])
        return x.reshape(NCORES, 128, BP, ST).transpose(0, 2, 3, 1).reshape(B, S)

    dot = unpack("od")
    ss = unpack("os")
    wd = (C_D * dot) / (D * np.sqrt(np.maximum(ss, 1e-20)))
    return np.ascontiguousarray(wd[:, None, :])


def _run(inputs, trace=False, **kw):
    nc = _get_nc()
    in_maps = _prep_inputs(inputs["p"], inputs["q"])
    res = run_bass_kernel_spmd(nc, in_maps, list(range(NCORES)), trace=trace, **kw)
    return _postprocess(res.results), res


def kernel(p, q):
    out, _ = _run({"p": p, "q": q})
    return out


# revision 15
# speedup vs baseline: 1.2333x; 1.2333x over previous
"""AttentiveMatch kernel for Trainium2 (8 NeuronCores, data-parallel over batch).

Reference math (per batch):
    pn = l2norm(p); qn = l2norm(q)
    w  = -(pn @ qn^T) / D          # [S,S]
    mv = (w @ q) / S               # [S,D]
    mn = l2norm(mv)
    out = -mean(pn * mn, -1)       # [S]

Rewritten with u_i = sum_j cos_ij q_j (signs cancel):
    out_i = (1/D) (pn_i . u_i) / |u_i|

Device pipeline (fp8 DoubleRow matmuls, all scalar folding done on host):
    mm1 [j,i]:  C' = (s2 q)^T-layout.T @ (s1 pn)^T-layout = s1 s2 |q_j| cos
    A8 = fp8(C' * G/(s1 s2))                 DVE tensor_scalar, const scale
    mm2 [i,d]:  M2 = A8-lhsT.T @ (s1 qn) = s1 G u    (i-partition layout)
    ss col  = accum_out of ACT Square(M2/256)        free-axis reduce
    dot col = accum_out of DVE (pn8 * M2) / 4096     tensor_tensor_reduce
Both land as [i%128, b*ST+it] columns; host does out = dot / (D sqrt(ss)).
Norms |p|,|q| folded into the host-side fp8 operands.
"""

import os
import sys

for _p in ("/opt/trn_rl_repo",):
    if _p not in sys.path:
        sys.path.append(_p)

import numpy as np
import ml_dtypes

import concourse.bacc as bacc
import concourse.mybir as mybir
import concourse.tile as tile
from concourse.bass_utils import run_bass_kernel_spmd

B, S, D = 64, 512, 768
NCORES = 8
BP = B // NCORES          # batches per core
ST = S // 128             # s tiles (4)
KT = D // 128             # d tiles (6)
F32 = mybir.dt.float32
BF16 = mybir.dt.bfloat16
F8 = mybir.dt.float8e4
AF = mybir.ActivationFunctionType
ALU = mybir.AluOpType
DR = mybir.MatmulPerfMode.DoubleRow

S1 = 16.0                 # fp8 pre-scale for pn/qn operands
S2 = 8.0                  # fp8 pre-scale for the raw-q operand
G = 16.0                  # fp8 A-matrix scale; S1*G = 256 so ss = |u|^2
C_A = G / (S1 * S2)       # PSUM -> A8
C_S = 1.0 / (S1 * G)      # M2 -> u
C_D = 1.0 / (S1 * S1 * G)  # (s1 pn)*(M2) -> pn.u

_NC = None

if os.environ.get("KERNEL_LDW_OPT", "0") == "1":
    import concourse.bass_utils as _bu

    _orig_run_command = _bu.run_command

    def _patched_run_command(cmd, **kw):
        cmd = [
            ("--enable-ldw-opt=true" if c == "--enable-ldw-opt=false" else c)
            for c in cmd
        ]
        return _orig_run_command(cmd, **kw)

    _bu.run_command = _patched_run_command


def _build():
    nc = bacc.Bacc("TRN2", target_bir_lowering=False, debug=False, num_devices=NCORES)
    qt_d = nc.dram_tensor("qt", [BP, 128, KT, S], F8, kind="ExternalInput")
    pt_d = nc.dram_tensor("pt", [BP, 128, KT, S], F8, kind="ExternalInput")
    qn_d = nc.dram_tensor("qn", [BP, 128, ST, D], F8, kind="ExternalInput")
    pn_d = nc.dram_tensor("pn", [BP, 128, ST, D], BF16, kind="ExternalInput")
    od_d = nc.dram_tensor("od", [128, BP * ST], F32, kind="ExternalOutput")
    os_d = nc.dram_tensor("os", [128, BP * ST], F32, kind="ExternalOutput")

    with tile.TileContext(nc) as tc:
        with (
            tc.tile_pool(name="cst", bufs=1) as cst,
            tc.tile_pool(name="inp", bufs=3) as inp,
            tc.tile_pool(name="ats", bufs=2) as ats,
            tc.tile_pool(name="gps", bufs=3, space="PSUM") as gps,
            tc.tile_pool(name="mps", bufs=2, space="PSUM") as mps,
            tc.tile_pool(name="scr", bufs=2) as scr,
        ):
            out_dot = cst.tile([128, BP * ST], F32)
            out_ss = cst.tile([128, BP * ST], F32)

            for b in range(BP):
                qt = inp.tile([128, KT, S], F8, tag="qt")
                pt = inp.tile([128, KT, S], F8, tag="pt")
                for c in range(KT // 2):
                    sl = slice(2 * c, 2 * c + 2)
                    nc.sync.dma_start(qt[:, sl, :], qt_d[b, :, sl, :])
                    nc.gpsimd.dma_start(pt[:, sl, :], pt_d[b, :, sl, :])
                qn = inp.tile([128, ST, D], F8, tag="qn")
                nc.gpsimd.dma_start(qn[:], qn_d[b])
                pnN = inp.tile([128, ST, D], BF16, tag="pn")
                nc.scalar.dma_start(pnN[:], pn_d[b])

                # mm1: C'[j,i] = sum_d (s2 q)[j,d] (s1 pn)[i,d]; A8 peel on
                # DVE with a constant scale (|q| folded into the qt operand).
                at_all = ats.tile([128, ST, S], F8, tag="at")
                for j in range(ST):
                    g_ps = gps.tile([128, S], F32, tag="g")
                    for c in range(KT // 2):
                        nc.tensor.matmul(
                            g_ps[:],
                            lhsT=qt[:, 2 * c:2 * c + 2, j * 128:(j + 1) * 128],
                            rhs=pt[:, 2 * c:2 * c + 2, :],
                            start=(c == 0), stop=(c == KT // 2 - 1),
                            perf_mode=DR,
                        )
                    nc.vector.tensor_scalar_mul(at_all[:, j, :], g_ps[:], C_A)

                # mm2 (i-partition layout): M2[i,d] = sum_j A8[j,i] (s1 qn)[j,d]
                # d split 512+256 to respect the one-bank matmul-out rule.
                # ss_i  = sum_d (M2/256)^2      ACT Square + accum_out
                # dot_i = sum_d pn8 M2 / 4096   DVE tensor_tensor_reduce
                # Both accum_outs land in the final [i%128, it] layout.
                sq = scr.tile([128, D], BF16, tag="sq")
                dp = scr.tile([128, D], BF16, tag="dp")
                for it in range(ST):
                    col = slice(b * ST + it, b * ST + it + 1)
                    m2 = mps.tile([128, 1024], F32, tag="m2")
                    lh = [at_all[:, 2 * c:2 * c + 2, it * 128:(it + 1) * 128]
                          for c in range(ST // 2)]
                    for lo, hi in ((0, 512), (512, 768)):
                        for c in range(ST // 2):
                            nc.tensor.matmul(
                                m2[:, lo:hi],
                                lhsT=lh[c],
                                rhs=qn[:, 2 * c:2 * c + 2, lo:hi],
                                start=(c == 0), stop=(c == ST // 2 - 1),
                                perf_mode=DR,
                            )
                    nc.scalar.activation(
                        sq[:], m2[:, 0:D], AF.Square, scale=C_S,
                        accum_out=out_ss[:, col])
                    nc.vector.scalar_tensor_tensor(
                        dp[:], pnN[:, it, :], C_D, m2[:, 0:D],
                        op0=ALU.mult, op1=ALU.mult,
                        accum_out=out_dot[:, col])

            nc.sync.dma_start(od_d[:], out_dot[:])
            nc.sync.dma_start(os_d[:], out_ss[:])
    nc.compile()
    return nc


def _get_nc():
    global _NC
    if _NC is None:
        _NC = _build()
    return _NC


def _prep_inputs(p, q):
    p = np.asarray(p, dtype=np.float32)
    q = np.asarray(q, dtype=np.float32)
    nq = np.sqrt(np.maximum((q * q).sum(-1), 1e-12))          # [B,S]
    npn = np.sqrt(np.maximum((p * p).sum(-1), 1e-12))
    q8 = (q * S2).astype(ml_dtypes.float8_e4m3)               # s2 * q
    pn8 = (p * (S1 / npn[..., None])).astype(ml_dtypes.float8_e4m3)
    qn8 = (q * (S1 / nq[..., None])).astype(ml_dtypes.float8_e4m3)

    # transposed: [core, b, part, kt, s] with d = kt*128 + part
    def tr(x):
        return np.ascontiguousarray(
            x.reshape(NCORES, BP, S, KT, 128).transpose(0, 1, 4, 3, 2)
        )

    # natural: [core, b, part, jt, d] with s = jt*128 + part
    def nat(x):
        return np.ascontiguousarray(
            x.reshape(NCORES, BP, ST, 128, D).transpose(0, 1, 3, 2, 4)
        )

    qt, pt = tr(q8), tr(pn8)
    qn_nat = nat(qn8)
    pn_nat = nat(pn8.astype(ml_dtypes.bfloat16))
    return [
        {"qt": qt[c], "pt": pt[c], "qn": qn_nat[c], "pn": pn_nat[c]}
        for c in range(NCORES)
    ]


def _postprocess(results):
    # [core, part, b*ST+it] -> batch core*BP+b, i = it*128 + part
    def unpack(key):
        x = np.stack([np.asarray(r[key], dtype=np.float32) for r in results])
        return x.reshape(NCORES, 128, BP, ST).transpose(0, 2, 3, 1).reshape(B, S)

    dot = unpack("od")
    ss = unpack("os")
    wd = dot / (D * np.sqrt(np.maximum(ss, 1e-20)))
    return np.ascontiguousarray(wd[:, None, :])


def _run(inputs, trace=False, **kw):
    nc = _get_nc()
    in_maps = _prep_inputs(inputs["p"], inputs["q"])
    res = run_bass_kernel_spmd(nc, in_maps, list(range(NCORES)), trace=trace, **kw)
    return _postprocess(res.results), res


def kernel(p, q):
    out, _ = _run({"p": p, "q": q})
    return out


# revision 16
# speedup vs baseline: 1.2563x; 1.0187x over previous
"""AttentiveMatch kernel for Trainium2 (8 NeuronCores, data-parallel over batch).

Reference math (per batch):
    pn = l2norm(p); qn = l2norm(q)
    w  = -(pn @ qn^T) / D          # [S,S]
    mv = (w @ q) / S               # [S,D]
    mn = l2norm(mv)
    out = -mean(pn * mn, -1)       # [S]

Rewritten with u_i = sum_j cos_ij q_j (signs cancel):
    out_i = (1/D) (pn_i . u_i) / |u_i|

Device pipeline (fp8 DoubleRow matmuls, all scalar folding done on host):
    mm1 [j,i]:  C' = (s2 q)^T-layout.T @ (s1 pn)^T-layout = s1 s2 |q_j| cos
    A8 = fp8(C' * G/(s1 s2))                 DVE tensor_scalar, const scale
    mm2 [i,d]:  M2 = A8-lhsT.T @ (s1 qn) = s1 G u    (i-partition layout)
    ss col  = accum_out of ACT Square(M2/256)        free-axis reduce
    dot col = accum_out of DVE (pn8 * M2) / 4096     tensor_tensor_reduce
Both land as [i%128, b*ST+it] columns; host does out = dot / (D sqrt(ss)).
Norms |p|,|q| folded into the host-side fp8 operands.
"""

import os
import sys

for _p in ("/opt/trn_rl_repo",):
    if _p not in sys.path:
        sys.path.append(_p)

import numpy as np
import ml_dtypes

import concourse.bacc as bacc
import concourse.mybir as mybir
import concourse.tile as tile
from concourse.bass_utils import run_bass_kernel_spmd

B, S, D = 64, 512, 768
NCORES = 8
BP = B // NCORES          # batches per core
ST = S // 128             # s tiles (4)
KT = D // 128             # d tiles (6)
F32 = mybir.dt.float32
BF16 = mybir.dt.bfloat16
F8 = mybir.dt.float8e4
AF = mybir.ActivationFunctionType
ALU = mybir.AluOpType
DR = mybir.MatmulPerfMode.DoubleRow

S1 = 16.0                 # fp8 pre-scale for pn/qn operands
S2 = 8.0                  # fp8 pre-scale for the raw-q operand
G = 16.0                  # fp8 A-matrix scale; S1*G = 256 so ss = |u|^2
C_A = G / (S1 * S2)       # PSUM -> A8
C_S = 1.0 / (S1 * G)      # M2 -> u
C_D = 1.0 / (S1 * S1 * G)  # (s1 pn)*(M2) -> pn.u

_NC = None

if os.environ.get("KERNEL_LDW_OPT", "0") == "1":
    import concourse.bass_utils as _bu

    _orig_run_command = _bu.run_command

    def _patched_run_command(cmd, **kw):
        cmd = [
            ("--enable-ldw-opt=true" if c == "--enable-ldw-opt=false" else c)
            for c in cmd
        ]
        return _orig_run_command(cmd, **kw)

    _bu.run_command = _patched_run_command


def _build():
    nc = bacc.Bacc("TRN2", target_bir_lowering=False, debug=False, num_devices=NCORES)
    qt_d = nc.dram_tensor("qt", [BP, 128, KT, S], F8, kind="ExternalInput")
    pt_d = nc.dram_tensor("pt", [BP, 128, KT, S], F8, kind="ExternalInput")
    qn_d = nc.dram_tensor("qn", [BP, 128, ST, D], F8, kind="ExternalInput")
    pn_d = nc.dram_tensor("pn", [BP, 128, ST, D], BF16, kind="ExternalInput")
    od_d = nc.dram_tensor("od", [128, BP * ST], F32, kind="ExternalOutput")
    os_d = nc.dram_tensor("os", [128, BP * ST], F32, kind="ExternalOutput")

    with tile.TileContext(nc) as tc:
        with (
            tc.tile_pool(name="cst", bufs=1) as cst,
            tc.tile_pool(name="inp", bufs=4) as inp,
            tc.tile_pool(name="ats", bufs=2) as ats,
            tc.tile_pool(name="gps", bufs=3, space="PSUM") as gps,
            tc.tile_pool(name="mps", bufs=2, space="PSUM") as mps,
            tc.tile_pool(name="scr", bufs=2) as scr,
        ):
            out_dot = cst.tile([128, BP * ST], F32)
            out_ss = cst.tile([128, BP * ST], F32)

            for b in range(BP):
                qt = inp.tile([128, KT, S], F8, tag="qt")
                pt = inp.tile([128, KT, S], F8, tag="pt")
                pt_ring = nc.scalar if b == 0 else nc.gpsimd
                for c in range(KT // 2):
                    sl = slice(2 * c, 2 * c + 2)
                    nc.sync.dma_start(qt[:, sl, :], qt_d[b, :, sl, :])
                    pt_ring.dma_start(pt[:, sl, :], pt_d[b, :, sl, :])
                qn = inp.tile([128, ST, D], F8, tag="qn")
                nc.gpsimd.dma_start(qn[:], qn_d[b])
                pnN = inp.tile([128, ST, D], BF16, tag="pn")
                nc.sync.dma_start(pnN[:], pn_d[b])

                # mm1: C'[j,i] = sum_d (s2 q)[j,d] (s1 pn)[i,d]; A8 peel on
                # DVE with a constant scale (|q| folded into the qt operand).
                at_all = ats.tile([128, ST, S], F8, tag="at")
                for j in range(ST):
                    g_ps = gps.tile([128, S], F32, tag="g")
                    for c in range(KT // 2):
                        nc.tensor.matmul(
                            g_ps[:],
                            lhsT=qt[:, 2 * c:2 * c + 2, j * 128:(j + 1) * 128],
                            rhs=pt[:, 2 * c:2 * c + 2, :],
                            start=(c == 0), stop=(c == KT // 2 - 1),
                            perf_mode=DR,
                        )
                    if j == 0:
                        nc.scalar.activation(at_all[:, j, :], g_ps[:],
                                             AF.Copy, scale=C_A)
                    else:
                        nc.vector.tensor_scalar_mul(at_all[:, j, :], g_ps[:],
                                                    C_A)

                # mm2 (i-partition layout): M2[i,d] = sum_j A8[j,i] (s1 qn)[j,d]
                # d split 512+256 to respect the one-bank matmul-out rule.
                # ss_i  = sum_d (M2/256)^2      ACT Square + accum_out
                # dot_i = sum_d pn8 M2 / 4096   DVE tensor_tensor_reduce
                # Both accum_outs land in the final [i%128, it] layout.
                sq = scr.tile([128, D], BF16, tag="sq")
                dp = scr.tile([128, D], BF16, tag="dp")
                for it in range(ST):
                    col = slice(b * ST + it, b * ST + it + 1)
                    m2 = mps.tile([128, 1024], F32, tag="m2")
                    lh = [at_all[:, 2 * c:2 * c + 2, it * 128:(it + 1) * 128]
                          for c in range(ST // 2)]
                    for lo, hi in ((0, 512), (512, 768)):
                        for c in range(ST // 2):
                            nc.tensor.matmul(
                                m2[:, lo:hi],
                                lhsT=lh[c],
                                rhs=qn[:, 2 * c:2 * c + 2, lo:hi],
                                start=(c == 0), stop=(c == ST // 2 - 1),
                                perf_mode=DR,
                            )
                    nc.scalar.activation(
                        sq[:], m2[:, 0:D], AF.Square, scale=C_S,
                        accum_out=out_ss[:, col])
                    nc.vector.scalar_tensor_tensor(
                        dp[:], pnN[:, it, :], C_D, m2[:, 0:D],
                        op0=ALU.mult, op1=ALU.mult,
                        accum_out=out_dot[:, col])

            nc.sync.dma_start(od_d[:], out_dot[:])
            nc.sync.dma_start(os_d[:], out_ss[:])
    nc.compile()
    return nc


def _get_nc():
    global _NC
    if _NC is None:
        _NC = _build()
    return _NC


def _prep_inputs(p, q):
    p = np.asarray(p, dtype=np.float32)
    q = np.asarray(q, dtype=np.float32)
    nq = np.sqrt(np.maximum((q * q).sum(-1), 1e-12))          # [B,S]
    npn = np.sqrt(np.maximum((p * p).sum(-1), 1e-12))
    q8 = (q * S2).astype(ml_dtypes.float8_e4m3)               # s2 * q
    pn8 = (p * (S1 / npn[..., None])).astype(ml_dtypes.float8_e4m3)
    qn8 = (q * (S1 / nq[..., None])).astype(ml_dtypes.float8_e4m3)

    # transposed: [core, b, part, kt, s] with d = kt*128 + part
    def tr(x):
        return np.ascontiguousarray(
            x.reshape(NCORES, BP, S, KT, 128).transpose(0, 1, 4, 3, 2)
        )

    # natural: [core, b, part, jt, d] with s = jt*128 + part
    def nat(x):
        return np.ascontiguousarray(
            x.reshape(NCORES, BP, ST, 128, D).transpose(0, 1, 3, 2, 4)
        )

    qt, pt = tr(q8), tr(pn8)
    qn_nat = nat(qn8)
    pn_nat = nat(pn8.astype(ml_dtypes.bfloat16))
    return [
        {"qt": qt[c], "pt": pt[c], "qn": qn_nat[c], "pn": pn_nat[c]}
        for c in range(NCORES)
    ]


def _postprocess(results):
    # [core, part, b*ST+it] -> batch core*BP+b, i = it*128 + part
    def unpack(key):
        x = np.stack([np.asarray(r[key], dtype=np.float32) for r in results])
        return x.reshape(NCORES, 128, BP, ST).transpose(0, 2, 3, 1).reshape(B, S)

    dot = unpack("od")
    ss = unpack("os")
    wd = dot / (D * np.sqrt(np.maximum(ss, 1e-20)))
    return np.ascontiguousarray(wd[:, None, :])


def _run(inputs, trace=False, **kw):
    nc = _get_nc()
    in_maps = _prep_inputs(inputs["p"], inputs["q"])
    res = run_bass_kernel_spmd(nc, in_maps, list(range(NCORES)), trace=trace, **kw)
    return _postprocess(res.results), res


def kernel(p, q):
    out, _ = _run({"p": p, "q": q})
    return out
